# revision 3
# baseline (speedup 1.0000x reference)
"""HeteroTripartiteGCN message passing on 8 Trainium2 NeuronCores.

Strategy: destination sharding (graph partitioning). Core k owns output rows
[k*N/8, (k+1)*N/8) of each node type, so each core computes its output slice
fully locally -- no cross-core reduction. Host buckets edges per
(core, source-chunk pass, dst tile of 128 rows), pads each bucket to a
multiple of 128 edges, and takes the cross-core max so one static program
serves all 8 cores (SPMD).

Per 128-edge chunk on device:
  M = dma_gather(x_src, src_idx)                  # [128 edges, 64] fp32
  S = (iota == dstloc) * val                      # DVE, fused one-hot*val
  psum[128 dst, 64] += S.T @ M                    # PE, exact segment-sum
Accumulators live in SBUF per (relation, dst tile). Weights are applied once
per dst tile at the end (linearity of segment_sum): out = relu(sum_rel
acc_rel @ W_rel), then DMA to per-core output slices.
"""
import sys

if '/opt/trn_rl_repo' not in sys.path:
    sys.path.insert(0, '/opt/trn_rl_repo')

import numpy as np

NU, NV, NF = 100000, 50000, 5000
D = 64
NCORES = 8
SRC_CHUNK = 25000          # gather index must fit int16 (<32768)
GATHER_TOKENS = 8192       # tokens per dma_gather call (64 chunks)


def rels(nu, nv, nf):
    # relations: (name, dst_count, src_count, x_src key, W key)
    return [
        ('uv', nu, nv, 'x_v', 'W_v_uv'),
        ('uf', nu, nf, 'x_f', 'W_f2u'),
        ('vu', nv, nu, 'x_u', 'W_u_uv'),
        ('vf', nv, nf, 'x_f', 'W_f2v'),
        ('fu', nf, nu, 'x_u', 'W_u2f'),
        ('fv', nf, nv, 'x_v', 'W_v2f'),
    ]


RELS = rels(NU, NV, NF)
# output types: (rel1, rel2) pairs feeding each dst type
TYPES = [('u', NU, 'uv', 'uf'), ('v', NV, 'vu', 'vf'), ('f', NF, 'fu', 'fv')]


def _ceil(a, b):
    return -(-a // b)


def preprocess(inputs):
    """Bucket/pad edges; build per-core device arrays + static schedule."""
    sched = {}
    percore = [dict() for _ in range(NCORES)]
    for name, n_dst, n_src, _xk, _wk in RELS:
        rows = np.asarray(inputs[name + '_rows']).ravel().astype(np.int64)
        cols = np.asarray(inputs[name + '_cols']).ravel().astype(np.int64)
        vals = np.asarray(inputs[name + '_vals']).ravel().astype(np.float32)
        shard = n_dst // NCORES
        ntile = _ceil(shard, 128)
        chunk = min(SRC_CHUNK, n_src)
        npass = _ceil(n_src, chunk)

        core = rows // shard
        loc = rows - core * shard
        tile = loc >> 7
        dstloc = (loc & 127).astype(np.float32)
        pss = cols // chunk
        srcloc = (cols - pss * chunk).astype(np.int16)

        key = (core * npass + pss) * ntile + tile
        order = np.argsort(key, kind='stable')
        key_s = key[order]
        counts = np.bincount(key, minlength=NCORES * npass * ntile)
        nch = _ceil(counts.reshape(NCORES, npass, ntile), 128)
        nch_max = nch.max(axis=0)                      # [npass, ntile]
        slots_pt = (nch_max * 128).astype(np.int64)
        seg_start = np.zeros(npass * ntile, np.int64)
        seg_start[1:] = np.cumsum(slots_pt.ravel())[:-1]
        tot = int(slots_pt.sum())

        seg_sorted_start = np.zeros(NCORES * npass * ntile, np.int64)
        seg_sorted_start[1:] = np.cumsum(counts)[:-1]
        rank = np.arange(len(key_s)) - seg_sorted_start[key_s]
        pos = seg_start[key_s % (npass * ntile)] + rank

        srcloc_s = srcloc[order]
        dstloc_s = dstloc[order]
        vals_s = vals[order]
        core_s = core[order]
        for c in range(NCORES):
            m = core_s == c
            sl = np.zeros(tot, np.int16)
            df = np.zeros(tot, np.float32)
            vf = np.zeros(tot, np.float32)
            pm = pos[m]
            sl[pm] = srcloc_s[m]
            df[pm] = dstloc_s[m]
            vf[pm] = vals_s[m]
            # device layouts
            gidx = np.tile(sl.reshape(tot // 16, 16).T, (8, 1)).copy()   # [128, tot/16]
            dstf = df.reshape(tot // 128, 128).T.copy()                  # [128, tot/128]
            valf = vf.reshape(tot // 128, 128).T.copy()
            percore[c][name] = (gidx, dstf, valf)
        sched[name] = dict(nch=nch_max, tot=tot, npass=npass, ntile=ntile,
                           chunk=chunk, n_src=n_src, shard=shard)
    return sched, percore


def build_bass(sched):
    import concourse.bacc as bacc
    from concourse import mybir
    from concourse.tile import TileContext

    nc = bacc.Bacc("TRN2", target_bir_lowering=False, debug=False,
                   num_devices=NCORES)
    f32 = mybir.dt.float32

    t_x = {k: nc.dram_tensor(k, [n, D], f32, kind="ExternalInput")
           for k, n in [('x_u', NU), ('x_v', NV), ('x_f', NF)]}
    t_w = {}
    for _name, _nd, _ns, _xk, wk in RELS:
        if wk not in t_w:
            t_w[wk] = nc.dram_tensor(wk, [D, D], f32, kind="ExternalInput")
    t_iota = nc.dram_tensor('iota', [128, 128], f32, kind="ExternalInput")
    t_ident = nc.dram_tensor('ident', [128, 128], f32, kind="ExternalInput")
    t_e = {}
    for name, _nd, _ns, _xk, _wk in RELS:
        tot = sched[name]['tot']
        t_e[name] = (
            nc.dram_tensor(name + '_gidx', [128, tot // 16], mybir.dt.int16,
                           kind="ExternalInput"),
            nc.dram_tensor(name + '_dstf', [128, tot // 128], f32,
                           kind="ExternalInput"),
            nc.dram_tensor(name + '_valf', [128, tot // 128], f32,
                           kind="ExternalInput"),
        )
    t_out = {}
    for tname, n_dst, _r1, _r2 in TYPES:
        ntile = _ceil(n_dst // NCORES, 128)
        t_out[tname] = nc.dram_tensor('out_' + tname, [ntile * 128, D], f32,
                                      kind="ExternalOutput")

    with TileContext(nc) as tc:
        # persistent SBUF: accumulators + constants
        acc = {}
        for name, n_dst, _ns, _xk, _wk in RELS:
            ntile = sched[name]['ntile']
            acc[name] = nc.alloc_sbuf_tensor('acc_' + name, [128, ntile * D], f32)
        s_iota = nc.alloc_sbuf_tensor('s_iota', [128, 128], f32)
        s_ident = nc.alloc_sbuf_tensor('s_ident', [128, 128], f32)
        s_w = {wk: nc.alloc_sbuf_tensor('s_' + wk, [D, D], f32) for wk in t_w}

        with tc.tile_pool(name="ld", bufs=1) as _ld:
            nc.sync.dma_start(s_iota[:, :], t_iota[:])
            nc.sync.dma_start(s_ident[:, :], t_ident[:])
            for wk in t_w:
                nc.sync.dma_start(s_w[wk][:, :], t_w[wk][:])

        # ---- phase 1: gather + one-hot matmul segment sums ----
        with tc.tile_pool(name="meta", bufs=3) as mpool, \
             tc.tile_pool(name="gath", bufs=3) as gpool, \
             tc.tile_pool(name="smat", bufs=4) as spool, \
             tc.tile_pool(name="psum", bufs=4, space="PSUM") as ppool:
            for name, _nd, _ns, xk, _wk in RELS:
                sc = sched[name]
                nch, npass, ntile, chunk = (sc['nch'], sc['npass'],
                                            sc['ntile'], sc['chunk'])
                col = 0  # global chunk column in dstf/valf for this rel
                # first pass with work per tile (copy vs add into acc)
                first_pass = np.full(ntile, -1)
                for p in range(npass):
                    for t in range(ntile):
                        if nch[p, t] > 0 and first_pass[t] < 0:
                            first_pass[t] = p
                for t in range(ntile):
                    if first_pass[t] < 0:   # no edges at all: zero acc tile
                        nc.vector.memset(acc[name][:, t * D:(t + 1) * D], 0.0)
                for p in range(npass):
                    # chunk list of this pass: (tile, first_of_tile, last_of_tile)
                    chunks = []
                    for t in range(ntile):
                        for j in range(nch[p, t]):
                            chunks.append((t, j == 0, j == nch[p, t] - 1))
                    nchunks = len(chunks)
                    if nchunks == 0:
                        continue
                    src_ap = t_x[xk][p * chunk:min(sc['n_src'], (p + 1) * chunk), :]
                    g0 = 0
                    p_cur = None
                    while g0 < nchunks:
                        gn = min(nchunks - g0, GATHER_TOKENS // 128)
                        ntok = gn * 128
                        c0 = col + g0
                        s_gidx = mpool.tile([128, ntok // 16], mybir.dt.int16,
                                            tag="gidx")
                        nc.sync.dma_start(
                            s_gidx[:], t_e[name][0][:, c0 * 8:(c0 + gn) * 8])
                        s_dstf = mpool.tile([128, gn], f32, tag="dstf")
                        nc.sync.dma_start(s_dstf[:], t_e[name][1][:, c0:c0 + gn])
                        s_valf = mpool.tile([128, gn], f32, tag="valf")
                        nc.sync.dma_start(s_valf[:], t_e[name][2][:, c0:c0 + gn])
                        s_m = gpool.tile([128, gn, D], f32, tag="m")
                        nc.gpsimd.dma_gather(
                            out_ap=s_m[:], in_ap=src_ap, idxs_ap=s_gidx[:],
                            num_idxs=ntok, num_idxs_reg=ntok, elem_size=D,
                            single_packet=False)
                        for j in range(gn):
                            tile_i, is_first, is_last = chunks[g0 + j]
                            s_S = spool.tile([128, 128], f32, tag="S")
                            nc.vector.tensor_scalar(
                                out=s_S[:], in0=s_iota[:, :],
                                scalar1=s_dstf[:, j:j + 1],
                                scalar2=s_valf[:, j:j + 1],
                                op0=mybir.AluOpType.is_equal,
                                op1=mybir.AluOpType.mult)
                            if is_first:
                                p_cur = ppool.tile([128, D], f32, tag="p")
                            nc.tensor.matmul(
                                out=p_cur[:], lhsT=s_S[:], rhs=s_m[:, j, :],
                                start=is_first, stop=is_last)
                            if is_last:
                                a_ap = acc[name][:, tile_i * D:(tile_i + 1) * D]
                                if p == first_pass[tile_i]:
                                    nc.vector.tensor_copy(a_ap, p_cur[:])
                                else:
                                    nc.vector.tensor_add(a_ap, a_ap, p_cur[:])
                        g0 += gn
                    col += nchunks

        # ---- phase 2: per-tile weight transform + relu + store ----
        with tc.tile_pool(name="ftp", bufs=4, space="PSUM") as ftp, \
             tc.tile_pool(name="fsb", bufs=4) as fsb, \
             tc.tile_pool(name="fop", bufs=2, space="PSUM") as fop, \
             tc.tile_pool(name="osb", bufs=3) as osb:
            for tname, n_dst, r1, r2 in TYPES:
                ntile = _ceil(n_dst // NCORES, 128)
                w1 = s_w[[r[4] for r in RELS if r[0] == r1][0]]
                w2 = s_w[[r[4] for r in RELS if r[0] == r2][0]]
                for t in range(ntile):
                    p_o = fop.tile([128, D], f32, tag="po")
                    for ri, (rel, w) in enumerate(((r1, w1), (r2, w2))):
                        a_ap = acc[rel][:, t * D:(t + 1) * D]
                        p_t = ftp.tile([D, 128], f32, tag="pt")
                        nc.tensor.transpose(p_t[:], a_ap, s_ident[:, :])
                        s_t = fsb.tile([D, 128], f32, tag="st")
                        nc.vector.tensor_copy(s_t[:], p_t[:])
                        nc.tensor.matmul(out=p_o[:], lhsT=s_t[:], rhs=w[:, :],
                                         start=(ri == 0), stop=(ri == 1))
                    o_t = osb.tile([128, D], f32, tag="o")
                    nc.scalar.activation(o_t[:], p_o[:],
                                         mybir.ActivationFunctionType.Relu)
                    nc.sync.dma_start(t_out[tname][t * 128:(t + 1) * 128, :],
                                      o_t[:])
    nc.compile()
    return nc


def make_inmaps(sched, percore, inputs):
    iota = np.tile(np.arange(128, dtype=np.float32), (128, 1))
    ident = np.eye(128, dtype=np.float32)
    base = {k: np.ascontiguousarray(np.asarray(inputs[k]), dtype=np.float32)
            for k in ['x_u', 'x_v', 'x_f',
                      'W_u_uv', 'W_v_uv', 'W_f2u', 'W_f2v', 'W_u2f', 'W_v2f']}
    in_maps = []
    for c in range(NCORES):
        im = dict(base)
        im['iota'] = iota
        im['ident'] = ident
        for name, _nd, _ns, _xk, _wk in RELS:
            g, d, v = percore[c][name]
            im[name + '_gidx'] = g
            im[name + '_dstf'] = d
            im[name + '_valf'] = v
        in_maps.append(im)
    return in_maps


def assemble(results):
    outs = []
    for tname, n_dst, _r1, _r2 in TYPES:
        shard = n_dst // NCORES
        outs.append(np.concatenate(
            [results[c]['out_' + tname][:shard] for c in range(NCORES)], axis=0))
    return tuple(outs)


def run(inputs, trace=False):
    from concourse.bass_utils import run_bass_kernel_spmd
    sched, percore = preprocess(inputs)
    nc = build_bass(sched)
    in_maps = make_inmaps(sched, percore, inputs)
    res = run_bass_kernel_spmd(nc, in_maps, core_ids=list(range(NCORES)),
                               trace=trace)
    return assemble(res.results), res


def kernel(**inputs):
    outs, _res = run(inputs, trace=False)
    return outs
